# revision 17
# baseline (speedup 1.0000x reference)
"""MLA attention kernel (DeepSeek-style) for 8 Trainium2 NeuronCores.

Sharding: core = b*4 + g*2 + t over (batch b in {0,1}) x (head-group g in
{0,1}: 8 heads each) x (query-fold t in {0,1}).  Each core runs attention
for its 8 heads and its 1024 query tokens (two causally-folded 512-blocks)
and a partial output projection; the host sums the two head-group partials.

The LoRA A-projections (q latent, kv latent + k_pe rope) are sharded
across the g-pair {(b,0,t), (b,1,t)} — both cores share the same token
layout, so each computes half the tokens and the halves are exchanged
with an AllGather over DRAM bounce buffers (runs on TOPSP/SDMA silicon,
overlapped with compute).  The host assigns each core's half via the
xqT / xkvT input columns, keeping the SPMD program uniform.  The KV
projection runs first so its AllGather hides under the Q projection, and
the per-head-pair main loop {k/v up-proj, qT up-proj, attention} keeps
the PE fed while the cq AllGather completes.

All tensors flow transposed ([feature-part, token-free]) so no on-chip
transposes are needed; causal structure is made SPMD-uniform by permuting
the key order per core and feeding full-block invalidation as per-partition
bias columns consumed by the Exp activation.  Matmul operands are fp16.

Partition-dim reductions (softmax denominator, RMS-norm square sums) use
ones[128,128] matmuls accumulating a broadcast [128,512] PSUM tile; the
8-chunk query block's denominator is accumulated on the Vector engine to
offload the PE.
"""

from contextlib import ExitStack

import numpy as np

import concourse.bacc as bacc
import concourse.bass as bass
import concourse.tile as tile
from concourse import mybir
from concourse.bass_utils import run_bass_kernel_spmd

# Problem shapes (hardcoded per contest contract)
B, S, D = 2, 2048, 2048
H = 16
QL = 1536  # q lora rank
KVL = 512  # kv lora rank
NOPE = 128
ROPE = 64
VD = 128
QKD = NOPE + ROPE  # 192
EPS = 1e-6
SCALE = QKD ** (-0.5)

HPC = 8         # heads per core
NQ = 1024       # query tokens per core
P = 128
NEG = -30000.0  # additive mask value (exp -> 0)

F32 = mybir.dt.float32
F16 = mybir.dt.float16
EXP = mybir.ActivationFunctionType.Exp
SQUARE = mybir.ActivationFunctionType.Square
SQRT = mybir.ActivationFunctionType.Sqrt

N_CORES = 8
SC_A = 8   # key 128-chunks for query block a
SC_B = 16  # key 128-chunks for query block b

ND = D // P        # 16
NRQ = QL // P      # 12
NRKV = KVL // P    # 4
HW = ROPE // 2     # 32

# g-pair groups (same batch, same query-fold, opposite head-group)
RG = [[0, 2], [1, 3], [4, 6], [5, 7]]

_CACHE = {}


def _rope(nc, pool, out_ap, ps, cos_ap, sin_ap, n):
    """rows 0:32 = even pair elems, 32:64 = odd.
    out[0:32] = e*cos - o*sin ; out[32:64] = e*sin + o*cos."""
    e = ps[0:HW, :]
    o = ps[HW:ROPE, :]
    t1 = pool.tile([HW, n], F32, tag="rp1", name="t1")
    nc.vector.tensor_mul(t1[:], e, cos_ap)
    t2 = pool.tile([HW, n], F32, tag="rp2", name="t2")
    nc.vector.tensor_mul(t2[:], o, sin_ap)
    nc.vector.tensor_sub(out_ap[0:HW, :], t1[:], t2[:])
    t3 = pool.tile([HW, n], F32, tag="rp3", name="t3")
    nc.vector.tensor_mul(t3[:], e, sin_ap)
    t4 = pool.tile([HW, n], F32, tag="rp4", name="t4")
    nc.vector.tensor_mul(t4[:], o, cos_ap)
    nc.vector.tensor_add(out_ap[HW:ROPE, :], t3[:], t4[:])


def build_nc():
    nc = bacc.Bacc("TRN2", target_bir_lowering=False, debug=False,
                   num_devices=N_CORES)

    def inp(name, shape, dt=F16):
        return nc.dram_tensor(name, shape, dt, kind="ExternalInput").ap()

    xqT = inp("xqT", [D, 512])
    xkvT = inp("xkvT", [D, NQ])
    wqa = inp("wq_a", [D, QL])
    wqbn = inp("wq_b_n", [QL, HPC * NOPE])
    wqbr = inp("wq_b_r", [QL, HPC * ROPE])
    wkva = inp("wkv_a", [D, KVL + ROPE])
    wkvbk = inp("wkv_b_k", [KVL, HPC * NOPE])
    wkvbv = inp("wkv_b_v", [KVL, HPC * VD])
    wo = inp("wo", [HPC * VD, D])
    cosq = inp("cosq", [HW, NQ], F32)
    sinq = inp("sinq", [HW, NQ], F32)
    cosk = inp("cosk", [HW, NQ], F32)
    sink = inp("sink", [HW, NQ], F32)
    bias_a = inp("bias_a", [P, SC_A], F32)
    bias_b = inp("bias_b", [P, SC_B], F32)
    out = nc.dram_tensor("out", [NQ, D], F32, kind="ExternalOutput").ap()

    with tile.TileContext(nc) as tc, ExitStack() as ctx, \
            nc.allow_low_precision(reason="fp16 matmul pipeline"):
        const = ctx.enter_context(tc.tile_pool(name="const", bufs=1))
        ones128 = const.tile([P, P], F16, tag="ones128")
        nc.vector.memset(ones128[:], 1.0)
        # multiplicative staircase masks (1 keep / 0 drop), applied post-exp
        stairs = []
        for j in range(4):
            st = const.tile([P, 512], F16, tag=f"stair{j}", name=f"st{j}")
            nc.vector.memset(st[:], 1.0)
            # keep 1 where f - p - 128j >= 0 else 0
            nc.gpsimd.affine_select(
                out=st[:], in_=st[:], compare_op=mybir.AluOpType.is_ge,
                fill=0.0, base=-128 * j, pattern=[[1, 512]],
                channel_multiplier=-1)
            stairs.append(st)
        bias_a_sb = const.tile([P, SC_A], F32, tag="bias_a")
        nc.sync.dma_start(bias_a_sb[:], bias_a[:])
        bias_b_sb = const.tile([P, SC_B], F32, tag="bias_b")
        nc.sync.dma_start(bias_b_sb[:], bias_b[:])
        eps_col = const.tile([P, 1], F32, tag="eps")
        nc.vector.memset(eps_col[:], EPS)

        # DRAM bounce buffers for the g-pair AllGathers
        dram = ctx.enter_context(tc.tile_pool(name="dram", bufs=1,
                                              space="DRAM"))
        cq_in = dram.tile([QL, 512], F16, tag="cq_in")
        cq_out = dram.tile([2 * QL, 512], F16, tag="cq_out")
        kv_in = dram.tile([KVL + ROPE, NQ], F16, tag="kv_in")
        kv_out = dram.tile([2 * (KVL + ROPE), NQ], F16, tag="kv_out")

        # persistent: kv latents + k_pe as two zero-padded K=128 variants
        latA = ctx.enter_context(tc.tile_pool(name="latA", bufs=1))
        kvT = [latA.tile([P, S], F16, tag=f"kvT{i}", name=f"kvT{i}")
               for i in range(NRKV)]
        kpe_e = latA.tile([P, S], F16, tag="kpe_e")
        kpe_o = latA.tile([P, S], F16, tag="kpe_o")
        nc.vector.memset(kpe_e[ROPE:P, :], 0.0)
        nc.vector.memset(kpe_o[0:ROPE, :], 0.0)
        kpez = (kpe_e, kpe_o)

        # packed cq latent: 24 [128,512] slices (rc, tbq) in 8 tiles;
        # reused as oTn after the up-projections
        latQ = ctx.enter_context(tc.tile_pool(name="latQ", bufs=1))
        cqPk = [latQ.tile([P, 1536], F16, tag=f"cqPk{i}", name=f"cqPk{i}")
                for i in range(8)]

        def cq_slice(rc, tbq):
            idx = rc * 2 + tbq
            t, c = idx // 3, (idx % 3) * 512
            return cqPk[t][:, c:c + 512]

        # dedicated attention-output tiles (cqPk stays live for the qT
        # chains of every head pair in the merged loop — no aliasing)
        otn_p = ctx.enter_context(tc.tile_pool(name="otn", bufs=1))
        oTn = [otn_p.tile([P, NQ], F16, tag=f"oTn{h}", name=f"oTn{h}")
               for h in range(HPC)]

        ps_main = ctx.enter_context(
            tc.tile_pool(name="ps_main", bufs=4, space="PSUM"))

        def mm_chain(ps_ap, pairs):
            n = len(pairs)
            for i, (lh, rh) in enumerate(pairs):
                nc.tensor.matmul(ps_ap, lh, rh,
                                 start=(i == 0), stop=(i == n - 1))

        ps_x = ctx.enter_context(
            tc.tile_pool(name="ps_x", bufs=2, space="PSUM"))

        # ---------- Phase 1: sharded A-projections + AllGathers ----------
        with ExitStack() as p1:
            tabk = p1.enter_context(tc.tile_pool(name="tabk", bufs=2))
            ropep = p1.enter_context(tc.tile_pool(name="ropep", bufs=2))
            wkva_p = p1.enter_context(tc.tile_pool(name="wkva", bufs=1))
            wqa_p = p1.enter_context(tc.tile_pool(name="wqa", bufs=1))
            wkva_sb = []
            for dc in range(ND):
                wt = wkva_p.tile([P, KVL + ROPE], F16, tag=f"wkva{dc}",
                                 name=f"wkva{dc}")
                nc.sync.dma_start(wt[:], wkva[dc * P:(dc + 1) * P, :])
                wkva_sb.append(wt)
            wqa_sb = []
            for dc in range(ND):
                wt = wqa_p.tile([P, QL], F16, tag=f"wqa{dc}",
                                name=f"wqa{dc}")
                nc.sync.dma_start(wt[:], wqa[dc * P:(dc + 1) * P, :])
                wqa_sb.append(wt)
            xt_p = p1.enter_context(tc.tile_pool(name="xt", bufs=2))
            sqp = p1.enter_context(tc.tile_pool(name="sq", bufs=3))

            def normalize(which, ntb):
                nrc, nfeat = ((NRKV, KVL) if which == 0 else (NRQ, QL))

                def sl_of(oc, tb):
                    if which == 0:
                        return kvT[oc][:, tb * 512:(tb + 1) * 512]
                    return cq_slice(oc, tb)
                for tb in range(ntb):
                    psb = ps_main.tile([P, 512], F32, tag="ps", name="psn")
                    for oc in range(nrc):
                        sq = sqp.tile([P, 512], F16, tag="sq", name="sq")
                        nc.scalar.activation(sq[:], sl_of(oc, tb), SQUARE)
                        nc.tensor.matmul(psb[:], ones128[:], sq[:],
                                         start=(oc == 0),
                                         stop=(oc == nrc - 1))
                    sd = sqp.tile([P, 512], F32, tag="sd", name="sd")
                    nc.scalar.activation(sd[:], psb[:], SQRT,
                                         bias=eps_col[:], scale=1.0 / nfeat)
                    rb = sqp.tile([P, 512], F32, tag="rb", name="rbn")
                    nc.vector.reciprocal_approx_fast(rb[:], sd[:])
                    for oc in range(nrc):
                        nc.vector.tensor_mul(sl_of(oc, tb), sl_of(oc, tb),
                                             rb[:])

            # KV A-proj first: this core's 1024 assigned tokens ->
            # kvT[:, 0:1024]; its AllGather hides under the Q projection.
            for hs in range(2):
                sl = slice(hs * 512, (hs + 1) * 512)
                xts = []
                for dc in range(ND):
                    xt = xt_p.tile([P, 512], F16, tag=f"xt{dc}",
                                   name=f"xt{dc}")
                    nc.sync.dma_start(xt[:], xkvT[dc * P:(dc + 1) * P, sl])
                    xts.append(xt)
                for oc in range(NRKV):
                    pool = ps_main if oc % 2 == 0 else ps_x
                    ps = pool.tile([P, 512], F32, tag="ps", name="ps1")
                    mm_chain(ps[:], [
                        (wkva_sb[dc][:, oc * P:(oc + 1) * P], xts[dc][:])
                        for dc in range(ND)])
                    if oc % 2 == 0:
                        nc.vector.tensor_copy(kvT[oc][:, sl], ps[:])
                    else:
                        nc.scalar.copy(kvT[oc][:, sl], ps[:])
                psp = ps_main.tile([ROPE, 512], F32, tag="ps", name="ps1p")
                mm_chain(psp[:], [
                    (wkva_sb[dc][:, KVL:KVL + ROPE], xts[dc][:])
                    for dc in range(ND)])
                ck = tabk.tile([HW, 512], F32, tag="cosk", name="ck")
                nc.sync.dma_start(ck[:], cosk[:, sl])
                sk = tabk.tile([HW, 512], F32, tag="sink", name="sk")
                nc.sync.dma_start(sk[:], sink[:, sl])
                _rope(nc, ropep, kpe_e[0:ROPE, sl], psp, ck[:], sk[:], 512)
            normalize(0, 2)
            # fire AG over the g-pair: kv latent + roped k_pe halves
            for oc in range(NRKV):
                nc.sync.dma_start(kv_in[oc * P:(oc + 1) * P, :],
                                  kvT[oc][:, 0:NQ])
            nc.sync.dma_start(kv_in[KVL:KVL + ROPE, :], kpe_e[0:ROPE, 0:NQ])
            nc.gpsimd.collective_compute(
                "AllGather", mybir.AluOpType.bypass, replica_groups=RG,
                ins=[kv_in[:].opt()], outs=[kv_out[:].opt()])

            # Q A-proj: this core's 512 assigned queries -> cq_slice(*, 0)
            xts = []
            for dc in range(ND):
                xt = xt_p.tile([P, 512], F16, tag=f"xt{dc}", name=f"xtq{dc}")
                nc.sync.dma_start(xt[:], xqT[dc * P:(dc + 1) * P, :])
                xts.append(xt)
            for oc in range(NRQ):
                pool = ps_main if oc % 2 == 0 else ps_x
                ps = pool.tile([P, 512], F32, tag="ps", name="ps1b")
                mm_chain(ps[:], [
                    (wqa_sb[dc][:, oc * P:(oc + 1) * P], xts[dc][:])
                    for dc in range(ND)])
                if oc % 2 == 0:
                    nc.vector.tensor_copy(cq_slice(oc, 0), ps[:])
                else:
                    nc.scalar.copy(cq_slice(oc, 0), ps[:])
            normalize(1, 1)
            # fire AG over the g-pair: cq halves
            for rc in range(NRQ):
                nc.sync.dma_start(cq_in[rc * P:(rc + 1) * P, :],
                                  cq_slice(rc, 0))
            nc.gpsimd.collective_compute(
                "AllGather", mybir.AluOpType.bypass, replica_groups=RG,
                ins=[cq_in[:].opt()], outs=[cq_out[:].opt()])

            # read back the gathered kv latent + k_pe (both halves)
            for oc in range(NRKV):
                nc.sync.dma_start(kvT[oc][:, 0:NQ],
                                  kv_out[oc * P:(oc + 1) * P, :])
                nc.sync.dma_start(
                    kvT[oc][:, NQ:S],
                    kv_out[KVL + ROPE + oc * P:KVL + ROPE + (oc + 1) * P, :])
            nc.sync.dma_start(kpe_e[0:ROPE, 0:NQ], kv_out[KVL:KVL + ROPE, :])
            nc.sync.dma_start(kpe_e[0:ROPE, NQ:S],
                              kv_out[2 * KVL + ROPE:2 * (KVL + ROPE), :])
            nc.sync.dma_start(kpe_o[ROPE:P, :], kpe_e[0:ROPE, :])

            # read back the gathered cq (rank0 half -> queries 0:512)
            for rc in range(NRQ):
                nc.sync.dma_start(cq_slice(rc, 0),
                                  cq_out[rc * P:(rc + 1) * P, :])
                nc.sync.dma_start(cq_slice(rc, 1),
                                  cq_out[QL + rc * P:QL + (rc + 1) * P, :])

        # ---------- Main loop: per head-pair {k/v up-proj, qT, attention} --
        with ExitStack() as p4:
            qt_p = p4.enter_context(tc.tile_pool(name="qt", bufs=2))
            tabq = p4.enter_context(tc.tile_pool(name="tabq", bufs=1))
            cq_sb = tabq.tile([HW, NQ], F32, tag="cosq")
            nc.sync.dma_start(cq_sb[:], cosq[:])
            sq_sb = tabq.tile([HW, NQ], F32, tag="sinq")
            nc.sync.dma_start(sq_sb[:], sinq[:])
            ropep3 = p4.enter_context(tc.tile_pool(name="ropep3", bufs=2))
            wqb_p = p4.enter_context(tc.tile_pool(name="wqb", bufs=2))
            kt_p = p4.enter_context(tc.tile_pool(name="kt", bufs=4))
            v_p = p4.enter_context(tc.tile_pool(name="v", bufs=2))
            wk_p = p4.enter_context(tc.tile_pool(name="wkvb", bufs=2))
            work = p4.enter_context(tc.tile_pool(name="work", bufs=2))
            accp = p4.enter_context(tc.tile_pool(name="accp", bufs=2))
            ptp = p4.enter_context(tc.tile_pool(name="ptp", bufs=28))
            ps_o = p4.enter_context(
                tc.tile_pool(name="ps_o", bufs=2, space="PSUM"))
            kTs = {}
            vts = {}

            def kv_up(hp2):
                heads2 = (2 * hp2, 2 * hp2 + 1)
                for h in heads2:
                    wk_sb = []
                    for rc in range(NRKV):
                        wt = wk_p.tile([P, NOPE], F16, tag=f"wkvbk{rc}",
                                       name=f"wkk{rc}")
                        nc.sync.dma_start(
                            wt[:], wkvbk[rc * P:(rc + 1) * P,
                                         h * NOPE:(h + 1) * NOPE])
                        wk_sb.append(wt)
                    kt = kt_p.tile([P, S], F16, tag="kt", name=f"kt{h}")
                    for tb in range(4):
                        sl = slice(tb * 512, (tb + 1) * 512)
                        ps = ps_main.tile([P, 512], F32, tag="ps",
                                          name="ps4k")
                        mm_chain(ps[:], [(wk_sb[rc][:], kvT[rc][:, sl])
                                         for rc in range(NRKV)])
                        if tb % 2 == 0:
                            nc.vector.tensor_copy(kt[:, sl], ps[:])
                        else:
                            nc.scalar.copy(kt[:, sl], ps[:])
                    kTs[h] = kt
                wv_sb = []
                for rc in range(NRKV):
                    wt = wk_p.tile([P, 2 * VD], F16, tag=f"wkvbv{rc}",
                                   name=f"wkv{rc}")
                    nc.sync.dma_start(
                        wt[:], wkvbv[rc * P:(rc + 1) * P,
                                     heads2[0] * VD:(heads2[0] + 2) * VD])
                    wv_sb.append(wt)
                vt = v_p.tile([P, 16 * 2 * VD], F16, tag="vt",
                              name=f"vt{hp2}")
                for tk in range(16):
                    ps = ps_main.tile([P, 2 * VD], F32, tag="ps",
                                      name="ps4v")
                    mm_chain(ps[:], [
                        (kvT[rc][:, tk * P:(tk + 1) * P], wv_sb[rc][:])
                        for rc in range(NRKV)])
                    if tk % 2 == 0:
                        nc.vector.tensor_copy(
                            vt[:, tk * 2 * VD:(tk + 1) * 2 * VD], ps[:])
                    else:
                        nc.scalar.copy(
                            vt[:, tk * 2 * VD:(tk + 1) * 2 * VD], ps[:])
                vts[hp2] = vt

            for hp in range(HPC // 2):
                heads = (2 * hp, 2 * hp + 1)
                # depth-2 pipeline: k/v up-proj for the NEXT pair is issued
                # before this pair's qT so the in-order PE queue has ready
                # work while the cq AllGather completes
                if hp == 0:
                    kv_up(0)
                    kv_up(1)
                elif hp < HPC // 2 - 1:
                    kv_up(hp + 1)

                # ---- qT up-projection for this head pair (paired rope) ----
                qtn = [qt_p.tile([P, NQ], F16, tag=f"qtn{i}",
                                 name=f"qtn{2 * hp + i}") for i in range(2)]
                qtp = qt_p.tile([P, NQ], F16, tag="qtp", name=f"qtp{hp}")
                wn, wr = [], []
                for rc in range(NRQ):
                    tn = wqb_p.tile([P, 2 * NOPE], F16, tag=f"wqbn{rc}",
                                    name=f"wqbn{rc}")
                    nc.sync.dma_start(
                        tn[:], wqbn[rc * P:(rc + 1) * P,
                                    hp * 2 * NOPE:(hp + 1) * 2 * NOPE])
                    wn.append(tn)
                    tr = wqb_p.tile([P, 2 * ROPE], F16, tag=f"wqbr{rc}",
                                    name=f"wqbr{rc}")
                    nc.sync.dma_start(
                        tr[:], wqbr[rc * P:(rc + 1) * P,
                                    hp * 2 * ROPE:(hp + 1) * 2 * ROPE])
                    wr.append(tr)
                for tbq in range(2):
                    sl = slice(tbq * 512, (tbq + 1) * 512)
                    for i in range(2):
                        h = 2 * hp + i
                        ps = ps_main.tile([P, 512], F32, tag="ps",
                                          name="ps3")
                        mm_chain(ps[:], [
                            (wn[rc][:, i * NOPE:(i + 1) * NOPE],
                             cq_slice(rc, tbq)) for rc in range(NRQ)])
                        if i == 0:
                            nc.vector.tensor_copy(qtn[i][:, sl], ps[:])
                        else:
                            nc.scalar.copy(qtn[i][:, sl], ps[:])
                    psp = ps_x.tile([P, 512], F32, tag="ps", name="ps3p")
                    mm_chain(psp[:], [(wr[rc][:], cq_slice(rc, tbq))
                                      for rc in range(NRQ)])
                    _rope(nc, ropep3, qtp[0:ROPE, sl], psp[0:ROPE, :],
                          cq_sb[:, sl], sq_sb[:, sl], 512)
                    _rope(nc, ropep3, qtp[ROPE:P, sl], psp[ROPE:P, :],
                          cq_sb[:, sl], sq_sb[:, sl], 512)

                # ---- attention for both heads ----
                for h in heads:
                    hv = h % 2
                    pts = {0: [], 1: []}
                    # qb=0 denominator on DVE, qb=1 accumulated on the PE
                    acc0 = accp.tile([P, 512], F16, tag="acc0",
                                     name=f"acc{h}")
                    psb1 = ps_x.tile([P, 512], F32, tag="ps",
                                     name=f"psd{h}")
                    for sc in range(SC_B):
                        # both query blocks share each stationary load
                        sps = {}
                        for qb in ((0, 1) if sc < SC_A else (1,)):
                            sps[qb] = ps_main.tile([P, 512], F32, tag="ps",
                                                   name="ps4s")
                            nc.tensor.matmul(
                                sps[qb][:], kTs[h][:, sc * P:(sc + 1) * P],
                                qtn[hv][:, qb * 512:qb * 512 + 512],
                                start=True, stop=False)
                        for qb in sps:
                            nc.tensor.matmul(
                                sps[qb][:],
                                kpez[hv][:, sc * P:(sc + 1) * P],
                                qtp[:, qb * 512:qb * 512 + 512],
                                start=False, stop=True)
                        for qb in sps:
                            nsc = SC_A if qb == 0 else SC_B
                            bias_sb = bias_a_sb if qb == 0 else bias_b_sb
                            pt = ptp.tile([P, 512], F16, tag="pt", name="pt")
                            jd = sc - (nsc - 4)
                            if jd >= 0:
                                nc.scalar.activation(pt[:], sps[qb][:], EXP)
                                nc.vector.tensor_mul(pt[:], pt[:],
                                                     stairs[jd][:])
                            else:
                                nc.scalar.activation(
                                    pt[:], sps[qb][:], EXP,
                                    bias=bias_sb[:, sc:sc + 1])
                            if qb == 0:
                                if sc == 0:
                                    nc.vector.tensor_copy(acc0[:], pt[:])
                                else:
                                    nc.vector.tensor_add(acc0[:], acc0[:],
                                                         pt[:])
                            else:
                                nc.tensor.matmul(psb1[:], ones128[:], pt[:],
                                                 start=(sc == 0),
                                                 stop=(sc == SC_B - 1))
                            pts[qb].append(pt)
                    oT = {qb: ps_o.tile([P, 512], F32, tag="oT",
                                        name=f"oT{qb}") for qb in (0, 1)}
                    for sc in range(SC_B):
                        for qb in ((0, 1) if sc < SC_A else (1,)):
                            nsc = SC_A if qb == 0 else SC_B
                            nc.tensor.matmul(
                                oT[qb][:],
                                vts[hp][:, sc * 2 * VD + hv * VD:
                                        sc * 2 * VD + (hv + 1) * VD],
                                pts[qb][sc][:], start=(sc == 0),
                                stop=(sc == nsc - 1))
                    psb0 = ps_main.tile([P, 512], F32, tag="ps", name="ps4d")
                    nc.tensor.matmul(psb0[:], ones128[:], acc0[:],
                                     start=True, stop=True)
                    for qb, psb in ((0, psb0), (1, psb1)):
                        rb = work.tile([P, 512], F32, tag="rb", name="rb")
                        nc.vector.reciprocal_approx_fast(rb[:], psb[:])
                        nc.vector.tensor_mul(
                            oTn[h][:, qb * 512:(qb + 1) * 512],
                            oT[qb][:], rb[:])

        # ---------- Phase 5: output projection (wo streamed per dcb) ------
        with ExitStack() as p5:
            os_p = p5.enter_context(tc.tile_pool(name="os", bufs=4))
            wos_p = p5.enter_context(tc.tile_pool(name="wos", bufs=2))
            for dcb in range(4):
                wos = []
                for h in range(HPC):
                    wt = wos_p.tile([P, 512], F16, tag=f"wos{h}",
                                    name=f"wos{h}")
                    nc.sync.dma_start(
                        wt[:], wo[h * P:(h + 1) * P,
                                  dcb * 512:(dcb + 1) * 512])
                    wos.append(wt)
                for tk in range(NQ // P):
                    ps = ps_main.tile([P, 512], F32, tag="ps", name="ps5")
                    for h in range(HPC):
                        nc.tensor.matmul(
                            ps[:], oTn[h][:, tk * P:(tk + 1) * P],
                            wos[h][:],
                            start=(h == 0), stop=(h == HPC - 1))
                    ot = os_p.tile([P, 512], F32, tag="ot", name="ot")
                    nc.scalar.copy(ot[:], ps[:])
                    nc.sync.dma_start(
                        out[tk * P:(tk + 1) * P,
                            dcb * 512:(dcb + 1) * 512], ot[:])

    nc.compile()
    return nc


def _prep_inputs(x, freqs_cis, wq_a, q_norm_w, wq_b, wkv_a, kv_norm_w,
                 wkv_b, wo):
    """Host-side shard prep. Returns (in_maps, meta) for 8 cores."""
    x = np.asarray(x, np.float32)
    freqs_cis = np.asarray(freqs_cis, np.float32)
    wq_a = np.asarray(wq_a, np.float32)
    q_norm_w = np.asarray(q_norm_w, np.float32)
    wq_b = np.asarray(wq_b, np.float32)
    wkv_a = np.asarray(wkv_a, np.float32)
    kv_norm_w = np.asarray(kv_norm_w, np.float32)
    wkv_b = np.asarray(wkv_b, np.float32)
    wo = np.asarray(wo, np.float32)

    f16 = np.float16
    # de-interleave perm for rope pairs: [e0..e31, o0..o31]
    perm = np.concatenate([np.arange(0, ROPE, 2), np.arange(1, ROPE, 2)])

    wqb = (wq_b * q_norm_w[:, None] * SCALE).reshape(QL, H, QKD)
    wqb_n = wqb[:, :, :NOPE].astype(f16)
    wqb_r = wqb[:, :, NOPE:][:, :, perm].astype(f16)

    wkva = np.ascontiguousarray(np.concatenate(
        [wkv_a[:, :KVL], wkv_a[:, KVL:][:, perm]], axis=1).astype(f16))

    wkvb = (wkv_b * kv_norm_w[:, None]).reshape(KVL, H, NOPE + VD).astype(f16)
    wkvb_k = wkvb[:, :, :NOPE]
    wkvb_v = wkvb[:, :, NOPE:]

    wqa16 = np.ascontiguousarray(wq_a.astype(f16))

    cos_t = np.ascontiguousarray(freqs_cis[:, :, 0].T)  # [32, S]
    sin_t = np.ascontiguousarray(freqs_cis[:, :, 1].T)

    sig0 = np.arange(S)
    sig1 = np.concatenate([sig0[512:1024], sig0[0:512],
                           sig0[1536:2048], sig0[1024:1536]])
    qpos = {0: np.concatenate([sig0[512:1024], sig0[1536:2048]]),
            1: np.concatenate([sig0[0:512], sig0[1024:1536]])}

    bias_a0 = np.zeros((P, SC_A), np.float32)
    bias_b0 = np.zeros((P, SC_B), np.float32)
    bias_a1 = np.zeros((P, SC_A), np.float32)
    bias_a1[:, 0:4] = NEG
    bias_b1 = np.zeros((P, SC_B), np.float32)
    bias_b1[:, 8:12] = NEG

    in_maps = []
    meta = []
    for c in range(N_CORES):
        b, g, t = c // 4, (c // 2) % 2, c % 2
        sig = sig0 if t == 0 else sig1
        qp = qpos[t]
        myq = qp[g * 512:(g + 1) * 512]      # assigned queries (global ids)
        mykv = sig[g * NQ:(g + 1) * NQ]      # assigned kv tokens
        hs = slice(g * HPC, (g + 1) * HPC)
        xb = x[b].T.astype(f16)
        m = {
            "xqT": np.ascontiguousarray(xb[:, myq]),
            "xkvT": np.ascontiguousarray(xb[:, mykv]),
            "wq_a": wqa16,
            "wq_b_n": np.ascontiguousarray(
                wqb_n[:, hs, :].reshape(QL, HPC * NOPE)),
            "wq_b_r": np.ascontiguousarray(
                wqb_r[:, hs, :].reshape(QL, HPC * ROPE)),
            "wkv_a": wkva,
            "wkv_b_k": np.ascontiguousarray(
                wkvb_k[:, hs, :].reshape(KVL, HPC * NOPE)),
            "wkv_b_v": np.ascontiguousarray(
                wkvb_v[:, hs, :].reshape(KVL, HPC * VD)),
            "wo": np.ascontiguousarray(
                wo[g * HPC * VD:(g + 1) * HPC * VD, :].astype(f16)),
            "cosq": np.ascontiguousarray(cos_t[:, qp]),
            "sinq": np.ascontiguousarray(sin_t[:, qp]),
            "cosk": np.ascontiguousarray(cos_t[:, mykv]),
            "sink": np.ascontiguousarray(sin_t[:, mykv]),
            "bias_a": bias_a0 if t == 0 else bias_a1,
            "bias_b": bias_b0 if t == 0 else bias_b1,
        }
        in_maps.append(m)
        meta.append((b, g, t))
    return in_maps, meta


def kernel(**inputs):
    in_maps, meta = _prep_inputs(**inputs)
    if "nc" not in _CACHE:
        _CACHE["nc"] = build_nc()
    nc = _CACHE["nc"]
    res = run_bass_kernel_spmd(nc, in_maps, core_ids=list(range(N_CORES)),
                               **_CACHE.get("run_kwargs", {}))
    _CACHE["last_result"] = res
    out = np.zeros((B, S, D), np.float32)
    for c in range(N_CORES):
        b, g, t = meta[c]
        part = res.results[c]["out"]  # [1024, 2048]
        if t == 0:
            out[b, 512:1024] += part[:512]
            out[b, 1536:2048] += part[512:]
        else:
            out[b, 0:512] += part[:512]
            out[b, 1024:1536] += part[512:]
    return out


# revision 40
# speedup vs baseline: 1.0273x; 1.0273x over previous
"""MLA attention kernel (DeepSeek-style) for 8 Trainium2 NeuronCores.

Sharding: core = b*4 + g*2 + t over (batch b in {0,1}) x (head-group g in
{0,1}: 8 heads each) x (query-fold t in {0,1}).  Each core runs attention
for its 8 heads and its 1024 query tokens (two causally-folded 512-blocks)
and a partial output projection; the host sums the two head-group partials.

The LoRA A-projections (q latent, kv latent + k_pe rope) are sharded
across the g-pair {(b,0,t), (b,1,t)} — both cores share the same token
layout, so each computes half the tokens and the halves are exchanged
with an AllGather over DRAM bounce buffers (runs on TOPSP/SDMA silicon,
overlapped with compute).  The host assigns each core's half via the
xqT / xkvT input columns, keeping the SPMD program uniform.

All tensors flow transposed ([feature-part, token-free]) so no on-chip
transposes are needed; causal structure is made SPMD-uniform by permuting
the key order per core and feeding full-block invalidation as per-partition
bias columns consumed by the Exp activation.  Matmul operands are fp16.

Partition-dim reductions (softmax denominator, RMS-norm square sums) use
ones[128,128] matmuls accumulating a broadcast [128,512] PSUM tile; the
8-chunk query block's denominator is accumulated on the Vector engine to
offload the PE.
"""

from contextlib import ExitStack

import numpy as np

import concourse.bacc as bacc
import concourse.bass as bass
import concourse.tile as tile
from concourse import mybir
from concourse.bass_utils import run_bass_kernel_spmd

# Problem shapes (hardcoded per contest contract)
B, S, D = 2, 2048, 2048
H = 16
QL = 1536  # q lora rank
KVL = 512  # kv lora rank
NOPE = 128
ROPE = 64
VD = 128
QKD = NOPE + ROPE  # 192
EPS = 1e-6
SCALE = QKD ** (-0.5)

HPC = 8         # heads per core
NQ = 1024       # query tokens per core
P = 128
NEG = -30000.0  # additive mask value (exp -> 0)

F32 = mybir.dt.float32
F16 = mybir.dt.float16
EXP = mybir.ActivationFunctionType.Exp
SQUARE = mybir.ActivationFunctionType.Square
SQRT = mybir.ActivationFunctionType.Sqrt

N_CORES = 8
SC_A = 8   # key 128-chunks for query block a
SC_B = 16  # key 128-chunks for query block b

ND = D // P        # 16
NRQ = QL // P      # 12
NRKV = KVL // P    # 4
HW = ROPE // 2     # 32

# g-pair groups (same batch, same query-fold, opposite head-group)
RG = [[0, 2], [1, 3], [4, 6], [5, 7]]

_CACHE = {}


def _rope(nc, pool, out_ap, ps, cos_ap, sin_ap, n):
    """rows 0:32 = even pair elems, 32:64 = odd.
    out[0:32] = e*cos - o*sin ; out[32:64] = e*sin + o*cos."""
    e = ps[0:HW, :]
    o = ps[HW:ROPE, :]
    t1 = pool.tile([HW, n], F32, tag="rp1", name="t1")
    nc.vector.tensor_mul(t1[:], e, cos_ap)
    t2 = pool.tile([HW, n], F32, tag="rp2", name="t2")
    nc.vector.tensor_mul(t2[:], o, sin_ap)
    nc.vector.tensor_sub(out_ap[0:HW, :], t1[:], t2[:])
    t3 = pool.tile([HW, n], F32, tag="rp3", name="t3")
    nc.vector.tensor_mul(t3[:], e, sin_ap)
    t4 = pool.tile([HW, n], F32, tag="rp4", name="t4")
    nc.vector.tensor_mul(t4[:], o, cos_ap)
    nc.vector.tensor_add(out_ap[HW:ROPE, :], t3[:], t4[:])


def build_nc():
    nc = bacc.Bacc("TRN2", target_bir_lowering=False, debug=False,
                   num_devices=N_CORES)

    def inp(name, shape, dt=F16):
        return nc.dram_tensor(name, shape, dt, kind="ExternalInput").ap()

    xqT = inp("xqT", [D, 512])
    xkvT = inp("xkvT", [D, NQ])
    wqa = inp("wq_a", [D, QL])
    wqbn = inp("wq_b_n", [QL, HPC * NOPE])
    wqbr = inp("wq_b_r", [QL, HPC * ROPE])
    wkva = inp("wkv_a", [D, KVL + ROPE])
    wkvbk = inp("wkv_b_k", [KVL, HPC * NOPE])
    wkvbv = inp("wkv_b_v", [KVL, HPC * VD])
    wo = inp("wo", [HPC * VD, D])
    cosq = inp("cosq", [HW, NQ], F32)
    sinq = inp("sinq", [HW, NQ], F32)
    cosk = inp("cosk", [HW, NQ], F32)
    sink = inp("sink", [HW, NQ], F32)
    bias_a = inp("bias_a", [P, SC_A], F32)
    bias_b = inp("bias_b", [P, SC_B], F32)
    out = nc.dram_tensor("out", [NQ, D], F32, kind="ExternalOutput").ap()

    with tile.TileContext(nc) as tc, ExitStack() as ctx, \
            nc.allow_low_precision(reason="fp16 matmul pipeline"):
        const = ctx.enter_context(tc.tile_pool(name="const", bufs=1))
        ones128 = const.tile([P, P], F16, tag="ones128")
        nc.vector.memset(ones128[:], 1.0)
        # multiplicative staircase masks (1 keep / 0 drop), applied post-exp
        stairs = []
        for j in range(4):
            st = const.tile([P, 512], F16, tag=f"stair{j}", name=f"st{j}")
            nc.vector.memset(st[:], 1.0)
            # keep 1 where f - p - 128j >= 0 else 0
            nc.gpsimd.affine_select(
                out=st[:], in_=st[:], compare_op=mybir.AluOpType.is_ge,
                fill=0.0, base=-128 * j, pattern=[[1, 512]],
                channel_multiplier=-1)
            stairs.append(st)
        bias_a_sb = const.tile([P, SC_A], F32, tag="bias_a")
        nc.sync.dma_start(bias_a_sb[:], bias_a[:])
        bias_b_sb = const.tile([P, SC_B], F32, tag="bias_b")
        nc.sync.dma_start(bias_b_sb[:], bias_b[:])
        eps_col = const.tile([P, 1], F32, tag="eps")
        nc.vector.memset(eps_col[:], EPS)

        # DRAM bounce buffers for the g-pair AllGathers
        dram = ctx.enter_context(tc.tile_pool(name="dram", bufs=1,
                                              space="DRAM"))
        cq_in = dram.tile([QL, 512], F16, tag="cq_in")
        cq_out = dram.tile([2 * QL, 512], F16, tag="cq_out")
        kv_in = dram.tile([KVL + ROPE, NQ], F16, tag="kv_in")
        kv_out = dram.tile([2 * (KVL + ROPE), NQ], F16, tag="kv_out")

        # persistent: kv latents + k_pe as two zero-padded K=128 variants
        latA = ctx.enter_context(tc.tile_pool(name="latA", bufs=1))
        kvT = [latA.tile([P, S], F16, tag=f"kvT{i}", name=f"kvT{i}")
               for i in range(NRKV)]
        kpe_e = latA.tile([P, S], F16, tag="kpe_e")
        kpe_o = latA.tile([P, S], F16, tag="kpe_o")
        nc.vector.memset(kpe_e[ROPE:P, :], 0.0)
        nc.vector.memset(kpe_o[0:ROPE, :], 0.0)
        kpez = (kpe_e, kpe_o)

        # packed cq latent: 24 [128,512] slices (rc, tbq) in 8 tiles;
        # reused as oTn after phase 3
        latQ = ctx.enter_context(tc.tile_pool(name="latQ", bufs=1))
        cqPk = [latQ.tile([P, 1536], F16, tag=f"cqPk{i}", name=f"cqPk{i}")
                for i in range(8)]

        def cq_slice(rc, tbq):
            idx = rc * 2 + tbq
            t, c = idx // 3, (idx % 3) * 512
            return cqPk[t][:, c:c + 512]

        oTn = [cqPk[h][:, 0:NQ] for h in range(HPC)]

        ps_main = ctx.enter_context(
            tc.tile_pool(name="ps_main", bufs=4, space="PSUM"))

        def mm_chain(ps_ap, pairs):
            n = len(pairs)
            for i, (lh, rh) in enumerate(pairs):
                nc.tensor.matmul(ps_ap, lh, rh,
                                 start=(i == 0), stop=(i == n - 1))

        ps_x_ctx = ExitStack()
        ps_x = ps_x_ctx.enter_context(
            tc.tile_pool(name="ps_x", bufs=2, space="PSUM"))

        # ---------- Phase 1: sharded A-projections + AllGathers ----------
        with ExitStack() as p1:
            tabk = p1.enter_context(tc.tile_pool(name="tabk", bufs=2))
            ropep = p1.enter_context(tc.tile_pool(name="ropep", bufs=2))
            wkva_p = p1.enter_context(tc.tile_pool(name="wkva", bufs=1))
            wqa_p = p1.enter_context(tc.tile_pool(name="wqa", bufs=1))
            wkva_sb = []
            for dc in range(ND):
                wt = wkva_p.tile([P, KVL + ROPE], F16, tag=f"wkva{dc}",
                                 name=f"wkva{dc}")
                nc.sync.dma_start(wt[:], wkva[dc * P:(dc + 1) * P, :])
                wkva_sb.append(wt)
            wqa_sb = []
            for dc in range(ND):
                wt = wqa_p.tile([P, QL], F16, tag=f"wqa{dc}",
                                name=f"wqa{dc}")
                nc.sync.dma_start(wt[:], wqa[dc * P:(dc + 1) * P, :])
                wqa_sb.append(wt)
            xt_p = p1.enter_context(tc.tile_pool(name="xt", bufs=2))
            sqp = p1.enter_context(tc.tile_pool(name="sq", bufs=3))

            def normalize(which, ntb):
                nrc, nfeat = ((NRKV, KVL) if which == 0 else (NRQ, QL))

                def sl_of(oc, tb):
                    if which == 0:
                        return kvT[oc][:, tb * 512:(tb + 1) * 512]
                    return cq_slice(oc, tb)
                for tb in range(ntb):
                    psb = ps_main.tile([P, 512], F32, tag="ps", name="psn")
                    for oc in range(nrc):
                        sq = sqp.tile([P, 512], F16, tag="sq", name="sq")
                        nc.scalar.activation(sq[:], sl_of(oc, tb), SQUARE)
                        nc.tensor.matmul(psb[:], ones128[:], sq[:],
                                         start=(oc == 0),
                                         stop=(oc == nrc - 1))
                    sd = sqp.tile([P, 512], F32, tag="sd", name="sd")
                    nc.scalar.activation(sd[:], psb[:], SQRT,
                                         bias=eps_col[:], scale=1.0 / nfeat)
                    rb = sqp.tile([P, 512], F32, tag="rb", name="rbn")
                    nc.vector.reciprocal_approx_fast(rb[:], sd[:])
                    for oc in range(nrc):
                        nc.vector.tensor_mul(sl_of(oc, tb), sl_of(oc, tb),
                                             rb[:])

            # KV A-proj first: this core's 1024 assigned tokens ->
            # kvT[:, 0:1024].  Its AllGather fires early so the hoisted
            # k/v up-projections can cover the cq AllGather afterwards.
            for hs in range(2):
                sl = slice(hs * 512, (hs + 1) * 512)
                xts = []
                for dc in range(ND):
                    xt = xt_p.tile([P, 512], F16, tag=f"xt{dc}",
                                   name=f"xt{dc}")
                    nc.sync.dma_start(xt[:], xkvT[dc * P:(dc + 1) * P, sl])
                    xts.append(xt)
                for oc in range(NRKV):
                    pool = ps_main if oc % 2 == 0 else ps_x
                    ps = pool.tile([P, 512], F32, tag="ps", name="ps1")
                    mm_chain(ps[:], [
                        (wkva_sb[dc][:, oc * P:(oc + 1) * P], xts[dc][:])
                        for dc in range(ND)])
                    if oc % 2 == 0:
                        nc.vector.tensor_copy(kvT[oc][:, sl], ps[:])
                    else:
                        nc.scalar.copy(kvT[oc][:, sl], ps[:])
                psp = ps_main.tile([ROPE, 512], F32, tag="ps", name="ps1p")
                mm_chain(psp[:], [
                    (wkva_sb[dc][:, KVL:KVL + ROPE], xts[dc][:])
                    for dc in range(ND)])
                ck = tabk.tile([HW, 512], F32, tag="cosk", name="ck")
                nc.sync.dma_start(ck[:], cosk[:, sl])
                sk = tabk.tile([HW, 512], F32, tag="sink", name="sk")
                nc.sync.dma_start(sk[:], sink[:, sl])
                _rope(nc, ropep, kpe_e[0:ROPE, sl], psp, ck[:], sk[:], 512)
            normalize(0, 2)
            # fire AG over the g-pair: kv latent + roped k_pe halves
            for oc in range(NRKV):
                nc.sync.dma_start(kv_in[oc * P:(oc + 1) * P, :],
                                  kvT[oc][:, 0:NQ])
            nc.sync.dma_start(kv_in[KVL:KVL + ROPE, :], kpe_e[0:ROPE, 0:NQ])
            nc.gpsimd.collective_compute(
                "AllGather", mybir.AluOpType.bypass, replica_groups=RG,
                ins=[kv_in[:].opt()], outs=[kv_out[:].opt()])

            # Q A-proj: this core's 512 assigned queries -> cq_slice(*, 0)
            xts = []
            for dc in range(ND):
                xt = xt_p.tile([P, 512], F16, tag=f"xt{dc}", name=f"xtq{dc}")
                nc.sync.dma_start(xt[:], xqT[dc * P:(dc + 1) * P, :])
                xts.append(xt)
            for oc in range(NRQ):
                pool = ps_main if oc % 2 == 0 else ps_x
                ps = pool.tile([P, 512], F32, tag="ps", name="ps1b")
                mm_chain(ps[:], [
                    (wqa_sb[dc][:, oc * P:(oc + 1) * P], xts[dc][:])
                    for dc in range(ND)])
                if oc % 2 == 0:
                    nc.vector.tensor_copy(cq_slice(oc, 0), ps[:])
                else:
                    nc.scalar.copy(cq_slice(oc, 0), ps[:])
            normalize(1, 1)
            # fire AG over the g-pair: cq halves (serializes after AG_kv on
            # the gpsimd queue; the hoisted k/v up-projections cover it)
            for rc in range(NRQ):
                nc.sync.dma_start(cq_in[rc * P:(rc + 1) * P, :],
                                  cq_slice(rc, 0))
            nc.gpsimd.collective_compute(
                "AllGather", mybir.AluOpType.bypass, replica_groups=RG,
                ins=[cq_in[:].opt()], outs=[cq_out[:].opt()])

            # read back the gathered kv latent + k_pe (both halves)
            for oc in range(NRKV):
                nc.sync.dma_start(kvT[oc][:, 0:NQ],
                                  kv_out[oc * P:(oc + 1) * P, :])
                nc.sync.dma_start(
                    kvT[oc][:, NQ:S],
                    kv_out[KVL + ROPE + oc * P:KVL + ROPE + (oc + 1) * P, :])
            nc.sync.dma_start(kpe_e[0:ROPE, 0:NQ], kv_out[KVL:KVL + ROPE, :])
            nc.sync.dma_start(kpe_e[0:ROPE, NQ:S],
                              kv_out[2 * KVL + ROPE:2 * (KVL + ROPE), :])
            nc.sync.dma_start(kpe_o[ROPE:P, :], kpe_e[0:ROPE, :])

        # ---- hoisted k/v up-projection (pairs 0,1): PE work that only
        # needs the kv AllGather, covering the cq AllGather's flight ----
        kt_p = ctx.enter_context(tc.tile_pool(name="kt", bufs=4))
        v_p = ctx.enter_context(tc.tile_pool(name="v", bufs=2))
        wk_p = ctx.enter_context(tc.tile_pool(name="wkvb", bufs=2))
        kTs = {}
        vts = {}

        def kv_up(hp2):
            heads2 = (2 * hp2, 2 * hp2 + 1)
            for h in heads2:
                wk_sb = []
                for rc in range(NRKV):
                    wt = wk_p.tile([P, NOPE], F16, tag=f"wkvbk{rc}",
                                   name=f"wkk{rc}")
                    nc.sync.dma_start(
                        wt[:], wkvbk[rc * P:(rc + 1) * P,
                                     h * NOPE:(h + 1) * NOPE])
                    wk_sb.append(wt)
                kt = kt_p.tile([P, S], F16, tag="kt", name=f"kt{h}")
                for tb in range(4):
                    sl = slice(tb * 512, (tb + 1) * 512)
                    ps = ps_main.tile([P, 512], F32, tag="ps", name="ps4k")
                    mm_chain(ps[:], [(wk_sb[rc][:], kvT[rc][:, sl])
                                     for rc in range(NRKV)])
                    if tb % 2 == 0:
                        nc.vector.tensor_copy(kt[:, sl], ps[:])
                    else:
                        nc.scalar.copy(kt[:, sl], ps[:])
                kTs[h] = kt
            wv_sb = []
            for rc in range(NRKV):
                wt = wk_p.tile([P, 2 * VD], F16, tag=f"wkvbv{rc}",
                               name=f"wkv{rc}")
                nc.sync.dma_start(
                    wt[:], wkvbv[rc * P:(rc + 1) * P,
                                 heads2[0] * VD:(heads2[0] + 2) * VD])
                wv_sb.append(wt)
            vt = v_p.tile([P, 16 * 2 * VD], F16, tag="vt", name=f"vt{hp2}")
            for tk in range(16):
                ps = ps_main.tile([P, 2 * VD], F32, tag="ps", name="ps4v")
                mm_chain(ps[:], [
                    (kvT[rc][:, tk * P:(tk + 1) * P], wv_sb[rc][:])
                    for rc in range(NRKV)])
                if tk % 2 == 0:
                    nc.vector.tensor_copy(
                        vt[:, tk * 2 * VD:(tk + 1) * 2 * VD], ps[:])
                else:
                    nc.scalar.copy(
                        vt[:, tk * 2 * VD:(tk + 1) * 2 * VD], ps[:])
            vts[hp2] = vt

        kv_up(0)
        kv_up(1)

        # read back the gathered cq (issued after the k-up weight loads so
        # its cq-AllGather wait doesn't block them on the in-order queue)
        for rc in range(NRQ):
            nc.sync.dma_start(cq_slice(rc, 0),
                              cq_out[rc * P:(rc + 1) * P, :])
            nc.sync.dma_start(cq_slice(rc, 1),
                              cq_out[QL + rc * P:QL + (rc + 1) * P, :])

        # ---------- Phase 3: qT for all heads (head-paired rope) ----------
        latQT = ctx.enter_context(tc.tile_pool(name="latQT", bufs=1))
        qTn = [latQT.tile([P, NQ], F16, tag=f"qTn{h}", name=f"qTn{h}")
               for h in range(HPC)]
        qTpk = [latQT.tile([P, NQ], F16, tag=f"qTpk{i}", name=f"qTpk{i}")
                for i in range(HPC // 2)]
        with ExitStack() as p3:
            tabq = p3.enter_context(tc.tile_pool(name="tabq", bufs=1))
            cq_sb = tabq.tile([HW, NQ], F32, tag="cosq")
            nc.sync.dma_start(cq_sb[:], cosq[:])
            sq_sb = tabq.tile([HW, NQ], F32, tag="sinq")
            nc.sync.dma_start(sq_sb[:], sinq[:])
            ropep3 = p3.enter_context(tc.tile_pool(name="ropep3", bufs=2))
            wqb_p = p3.enter_context(tc.tile_pool(name="wqb", bufs=3))
            for hp in range(HPC // 2):
                wn, wr = [], []
                for rc in range(NRQ):
                    tn = wqb_p.tile([P, 2 * NOPE], F16, tag=f"wqbn{rc}",
                                    name=f"wqbn{rc}")
                    nc.sync.dma_start(
                        tn[:], wqbn[rc * P:(rc + 1) * P,
                                    hp * 2 * NOPE:(hp + 1) * 2 * NOPE])
                    wn.append(tn)
                    tr = wqb_p.tile([P, 2 * ROPE], F16, tag=f"wqbr{rc}",
                                    name=f"wqbr{rc}")
                    nc.sync.dma_start(
                        tr[:], wqbr[rc * P:(rc + 1) * P,
                                    hp * 2 * ROPE:(hp + 1) * 2 * ROPE])
                    wr.append(tr)
                for tbq in range(2):
                    sl = slice(tbq * 512, (tbq + 1) * 512)
                    for i in range(2):
                        h = 2 * hp + i
                        ps = ps_main.tile([P, 512], F32, tag="ps",
                                          name="ps3")
                        mm_chain(ps[:], [
                            (wn[rc][:, i * NOPE:(i + 1) * NOPE],
                             cq_slice(rc, tbq)) for rc in range(NRQ)])
                        if i == 0:
                            nc.vector.tensor_copy(qTn[h][:, sl], ps[:])
                        else:
                            nc.scalar.copy(qTn[h][:, sl], ps[:])
                    psp = ps_x.tile([P, 512], F32, tag="ps", name="ps3p")
                    mm_chain(psp[:], [(wr[rc][:], cq_slice(rc, tbq))
                                      for rc in range(NRQ)])
                    _rope(nc, ropep3, qTpk[hp][0:ROPE, sl], psp[0:ROPE, :],
                          cq_sb[:, sl], sq_sb[:, sl], 512)
                    _rope(nc, ropep3, qTpk[hp][ROPE:P, sl], psp[ROPE:P, :],
                          cq_sb[:, sl], sq_sb[:, sl], 512)

        ps_x_ctx.close()

        # ---------- Phase 4: attention per head-pair ----------
        wo_p = ctx.enter_context(tc.tile_pool(name="wo", bufs=1))
        wo_sb = []
        for h in range(HPC):
            wt = wo_p.tile([P, D], F16, tag=f"wo{h}", name=f"wo{h}")
            nc.sync.dma_start(wt[:], wo[h * P:(h + 1) * P, :])
            wo_sb.append(wt)
        with ExitStack() as p4:
            work = p4.enter_context(tc.tile_pool(name="work", bufs=4))
            accp = p4.enter_context(tc.tile_pool(name="accp", bufs=4))
            ptp = p4.enter_context(tc.tile_pool(name="ptp", bufs=28))
            ps_o = p4.enter_context(
                tc.tile_pool(name="ps_o", bufs=2, space="PSUM"))
            ps_d = p4.enter_context(
                tc.tile_pool(name="ps_d", bufs=2, space="PSUM"))
            for hp in range(HPC // 2):
                heads = (2 * hp, 2 * hp + 1)
                for h in heads:
                    hv = h % 2
                    pts = {0: [], 1: []}
                    # qb=0 denominator on DVE, qb=1 accumulated on the PE
                    acc0 = accp.tile([P, 512], F16, tag="acc0",
                                     name=f"acc{h}")
                    psb1 = ps_d.tile([P, 512], F32, tag="psd",
                                     name=f"psd{h}")
                    for sc in range(SC_B):
                        # both query blocks share each stationary load
                        sps = {}
                        for qb in ((0, 1) if sc < SC_A else (1,)):
                            sps[qb] = ps_main.tile([P, 512], F32, tag="ps",
                                                   name="ps4s")
                            nc.tensor.matmul(
                                sps[qb][:], kTs[h][:, sc * P:(sc + 1) * P],
                                qTn[h][:, qb * 512:qb * 512 + 512],
                                start=True, stop=False)
                        for qb in sps:
                            nc.tensor.matmul(
                                sps[qb][:],
                                kpez[hv][:, sc * P:(sc + 1) * P],
                                qTpk[h // 2][:, qb * 512:qb * 512 + 512],
                                start=False, stop=True)
                        for qb in sps:
                            nsc = SC_A if qb == 0 else SC_B
                            bias_sb = bias_a_sb if qb == 0 else bias_b_sb
                            pt = ptp.tile([P, 512], F16, tag="pt", name="pt")
                            jd = sc - (nsc - 4)
                            if jd >= 0:
                                nc.scalar.activation(pt[:], sps[qb][:], EXP)
                                nc.vector.tensor_mul(pt[:], pt[:],
                                                     stairs[jd][:])
                            else:
                                nc.scalar.activation(
                                    pt[:], sps[qb][:], EXP,
                                    bias=bias_sb[:, sc:sc + 1])
                            if qb == 0:
                                if sc == 0:
                                    nc.vector.tensor_copy(acc0[:], pt[:])
                                else:
                                    nc.vector.tensor_add(acc0[:], acc0[:],
                                                         pt[:])
                            else:
                                nc.tensor.matmul(psb1[:], ones128[:], pt[:],
                                                 start=(sc == 0),
                                                 stop=(sc == SC_B - 1))
                            pts[qb].append(pt)
                    oT = {qb: ps_o.tile([P, 512], F32, tag="oT",
                                        name=f"oT{qb}") for qb in (0, 1)}
                    for sc in range(SC_B):
                        for qb in ((0, 1) if sc < SC_A else (1,)):
                            nsc = SC_A if qb == 0 else SC_B
                            nc.tensor.matmul(
                                oT[qb][:],
                                vts[hp][:, sc * 2 * VD + hv * VD:
                                        sc * 2 * VD + (hv + 1) * VD],
                                pts[qb][sc][:], start=(sc == 0),
                                stop=(sc == nsc - 1))
                    psb0 = ps_main.tile([P, 512], F32, tag="ps", name="ps4d")
                    nc.tensor.matmul(psb0[:], ones128[:], acc0[:],
                                     start=True, stop=True)
                    for qb, psb in ((0, psb0), (1, psb1)):
                        rb = work.tile([P, 512], F32, tag="rb", name="rb")
                        nc.vector.reciprocal_approx_fast(rb[:], psb[:])
                        nc.vector.tensor_mul(
                            oTn[h][:, qb * 512:(qb + 1) * 512],
                            oT[qb][:], rb[:])
                # keep the in-order PE queue fed: compute the pair-(hp+2)
                # k/v up-projection between attention pairs
                if hp + 2 < HPC // 2:
                    kv_up(hp + 2)

        # ---------- Phase 5: output projection (wo aliases qTn/kvT) -------
        with ExitStack() as p5:
            os_p = p5.enter_context(tc.tile_pool(name="os", bufs=4))
            for tk in range(NQ // P):
                for dcb in range(4):
                    ps = ps_main.tile([P, 512], F32, tag="ps", name="ps5")
                    for h in range(HPC):
                        rh = wo_sb[h][:, dcb * 512:(dcb + 1) * 512]
                        nc.tensor.matmul(
                            ps[:], oTn[h][:, tk * P:(tk + 1) * P], rh,
                            start=(h == 0), stop=(h == HPC - 1))
                    ot = os_p.tile([P, 512], F32, tag="ot", name="ot")
                    nc.scalar.copy(ot[:], ps[:])
                    nc.sync.dma_start(
                        out[tk * P:(tk + 1) * P,
                            dcb * 512:(dcb + 1) * 512], ot[:])

    nc.compile()
    return nc


def _prep_inputs(x, freqs_cis, wq_a, q_norm_w, wq_b, wkv_a, kv_norm_w,
                 wkv_b, wo):
    """Host-side shard prep. Returns (in_maps, meta) for 8 cores."""
    x = np.asarray(x, np.float32)
    freqs_cis = np.asarray(freqs_cis, np.float32)
    wq_a = np.asarray(wq_a, np.float32)
    q_norm_w = np.asarray(q_norm_w, np.float32)
    wq_b = np.asarray(wq_b, np.float32)
    wkv_a = np.asarray(wkv_a, np.float32)
    kv_norm_w = np.asarray(kv_norm_w, np.float32)
    wkv_b = np.asarray(wkv_b, np.float32)
    wo = np.asarray(wo, np.float32)

    f16 = np.float16
    # de-interleave perm for rope pairs: [e0..e31, o0..o31]
    perm = np.concatenate([np.arange(0, ROPE, 2), np.arange(1, ROPE, 2)])

    wqb = (wq_b * q_norm_w[:, None] * SCALE).reshape(QL, H, QKD)
    wqb_n = wqb[:, :, :NOPE].astype(f16)
    wqb_r = wqb[:, :, NOPE:][:, :, perm].astype(f16)

    wkva = np.ascontiguousarray(np.concatenate(
        [wkv_a[:, :KVL], wkv_a[:, KVL:][:, perm]], axis=1).astype(f16))

    wkvb = (wkv_b * kv_norm_w[:, None]).reshape(KVL, H, NOPE + VD).astype(f16)
    wkvb_k = wkvb[:, :, :NOPE]
    wkvb_v = wkvb[:, :, NOPE:]

    wqa16 = np.ascontiguousarray(wq_a.astype(f16))

    cos_t = np.ascontiguousarray(freqs_cis[:, :, 0].T)  # [32, S]
    sin_t = np.ascontiguousarray(freqs_cis[:, :, 1].T)

    sig0 = np.arange(S)
    sig1 = np.concatenate([sig0[512:1024], sig0[0:512],
                           sig0[1536:2048], sig0[1024:1536]])
    qpos = {0: np.concatenate([sig0[512:1024], sig0[1536:2048]]),
            1: np.concatenate([sig0[0:512], sig0[1024:1536]])}

    bias_a0 = np.zeros((P, SC_A), np.float32)
    bias_b0 = np.zeros((P, SC_B), np.float32)
    bias_a1 = np.zeros((P, SC_A), np.float32)
    bias_a1[:, 0:4] = NEG
    bias_b1 = np.zeros((P, SC_B), np.float32)
    bias_b1[:, 8:12] = NEG

    in_maps = []
    meta = []
    for c in range(N_CORES):
        b, g, t = c // 4, (c // 2) % 2, c % 2
        sig = sig0 if t == 0 else sig1
        qp = qpos[t]
        myq = qp[g * 512:(g + 1) * 512]      # assigned queries (global ids)
        mykv = sig[g * NQ:(g + 1) * NQ]      # assigned kv tokens
        hs = slice(g * HPC, (g + 1) * HPC)
        xb = x[b].T.astype(f16)
        m = {
            "xqT": np.ascontiguousarray(xb[:, myq]),
            "xkvT": np.ascontiguousarray(xb[:, mykv]),
            "wq_a": wqa16,
            "wq_b_n": np.ascontiguousarray(
                wqb_n[:, hs, :].reshape(QL, HPC * NOPE)),
            "wq_b_r": np.ascontiguousarray(
                wqb_r[:, hs, :].reshape(QL, HPC * ROPE)),
            "wkv_a": wkva,
            "wkv_b_k": np.ascontiguousarray(
                wkvb_k[:, hs, :].reshape(KVL, HPC * NOPE)),
            "wkv_b_v": np.ascontiguousarray(
                wkvb_v[:, hs, :].reshape(KVL, HPC * VD)),
            "wo": np.ascontiguousarray(
                wo[g * HPC * VD:(g + 1) * HPC * VD, :].astype(f16)),
            "cosq": np.ascontiguousarray(cos_t[:, qp]),
            "sinq": np.ascontiguousarray(sin_t[:, qp]),
            "cosk": np.ascontiguousarray(cos_t[:, mykv]),
            "sink": np.ascontiguousarray(sin_t[:, mykv]),
            "bias_a": bias_a0 if t == 0 else bias_a1,
            "bias_b": bias_b0 if t == 0 else bias_b1,
        }
        in_maps.append(m)
        meta.append((b, g, t))
    return in_maps, meta


def kernel(**inputs):
    in_maps, meta = _prep_inputs(**inputs)
    if "nc" not in _CACHE:
        _CACHE["nc"] = build_nc()
    nc = _CACHE["nc"]
    res = run_bass_kernel_spmd(nc, in_maps, core_ids=list(range(N_CORES)),
                               **_CACHE.get("run_kwargs", {}))
    _CACHE["last_result"] = res
    out = np.zeros((B, S, D), np.float32)
    for c in range(N_CORES):
        b, g, t = meta[c]
        part = res.results[c]["out"]  # [1024, 2048]
        if t == 0:
            out[b, 512:1024] += part[:512]
            out[b, 1536:2048] += part[512:]
        else:
            out[b, 0:512] += part[:512]
            out[b, 1024:1536] += part[512:]
    return out


# revision 43
# speedup vs baseline: 1.1636x; 1.1327x over previous
"""MLA attention kernel (DeepSeek-style) for 8 Trainium2 NeuronCores.

Sharding: core = b*4 + g*2 + t over (batch b in {0,1}) x (head-group g in
{0,1}: 8 heads each) x (query-fold t in {0,1}).  Each core runs attention
for its 8 heads and its 1024 query tokens (two causally-folded 512-blocks)
and a partial output projection; the host sums the two head-group partials.

The LoRA A-projections (q latent, kv latent + k_pe rope) are sharded
across the g-pair {(b,0,t), (b,1,t)} — both cores share the same token
layout, so each computes half the tokens and the halves are exchanged
with an AllGather over DRAM bounce buffers (runs on TOPSP/SDMA silicon,
overlapped with compute).  The host assigns each core's half via the
xqT / xkvT input columns, keeping the SPMD program uniform.

All tensors flow transposed ([feature-part, token-free]) so no on-chip
transposes are needed; causal structure is made SPMD-uniform by permuting
the key order per core and feeding full-block invalidation as per-partition
bias columns consumed by the Exp activation.  Matmul operands are fp16.

Partition-dim reductions (softmax denominator, RMS-norm square sums) use
ones[128,128] matmuls accumulating a broadcast [128,512] PSUM tile; the
8-chunk query block's denominator is accumulated on the Vector engine to
offload the PE.
"""

from contextlib import ExitStack

import numpy as np

import concourse.bacc as bacc
import concourse.bass as bass
import concourse.tile as tile
from concourse import mybir
from concourse.bass_utils import run_bass_kernel_spmd

# Problem shapes (hardcoded per contest contract)
B, S, D = 2, 2048, 2048
H = 16
QL = 1536  # q lora rank
KVL = 512  # kv lora rank
NOPE = 128
ROPE = 64
VD = 128
QKD = NOPE + ROPE  # 192
EPS = 1e-6
SCALE = QKD ** (-0.5)

HPC = 8         # heads per core
NQ = 1024       # query tokens per core
P = 128
NEG = -30000.0  # additive mask value (exp -> 0)

F32 = mybir.dt.float32
F16 = mybir.dt.float16
EXP = mybir.ActivationFunctionType.Exp
SQUARE = mybir.ActivationFunctionType.Square
SQRT = mybir.ActivationFunctionType.Sqrt

N_CORES = 8
SC_A = 8   # key 128-chunks for query block a
SC_B = 16  # key 128-chunks for query block b

ND = D // P        # 16
NRQ = QL // P      # 12
NRKV = KVL // P    # 4
HW = ROPE // 2     # 32

# g-pair groups (same batch, same query-fold, opposite head-group)
RG = [[0, 2], [1, 3], [4, 6], [5, 7]]

_CACHE = {}


def _rope(nc, pool, out_ap, ps, cos_ap, sin_ap, n):
    """rows 0:32 = even pair elems, 32:64 = odd.
    out[0:32] = e*cos - o*sin ; out[32:64] = e*sin + o*cos."""
    e = ps[0:HW, :]
    o = ps[HW:ROPE, :]
    t1 = pool.tile([HW, n], F32, tag="rp1", name="t1")
    nc.vector.tensor_mul(t1[:], e, cos_ap)
    t2 = pool.tile([HW, n], F32, tag="rp2", name="t2")
    nc.vector.tensor_mul(t2[:], o, sin_ap)
    nc.vector.tensor_sub(out_ap[0:HW, :], t1[:], t2[:])
    t3 = pool.tile([HW, n], F32, tag="rp3", name="t3")
    nc.vector.tensor_mul(t3[:], e, sin_ap)
    t4 = pool.tile([HW, n], F32, tag="rp4", name="t4")
    nc.vector.tensor_mul(t4[:], o, cos_ap)
    nc.vector.tensor_add(out_ap[HW:ROPE, :], t3[:], t4[:])


def build_nc():
    nc = bacc.Bacc("TRN2", target_bir_lowering=False, debug=False,
                   num_devices=N_CORES)

    def inp(name, shape, dt=F16):
        return nc.dram_tensor(name, shape, dt, kind="ExternalInput").ap()

    xqT = inp("xqT", [D, 512])
    xkvT = inp("xkvT", [D, NQ])
    wqa = inp("wq_a", [D, QL])
    wqbn = inp("wq_b_n", [QL, HPC * NOPE])
    wqbr = inp("wq_b_r", [QL, HPC * ROPE])
    wkva = inp("wkv_a", [D, KVL + ROPE])
    wkvbk = inp("wkv_b_k", [KVL, HPC * NOPE])
    wkvbv = inp("wkv_b_v", [KVL, HPC * VD])
    wo = inp("wo", [HPC * VD, D])
    cosq = inp("cosq", [HW, NQ], F32)
    sinq = inp("sinq", [HW, NQ], F32)
    cosk = inp("cosk", [HW, NQ], F32)
    sink = inp("sink", [HW, NQ], F32)
    bias_a = inp("bias_a", [P, SC_A], F32)
    bias_b = inp("bias_b", [P, SC_B], F32)
    out = nc.dram_tensor("out", [NQ, D], F32, kind="ExternalOutput").ap()

    with tile.TileContext(nc) as tc, ExitStack() as ctx, \
            nc.allow_low_precision(reason="fp16 matmul pipeline"):
        const = ctx.enter_context(tc.tile_pool(name="const", bufs=1))
        ones128 = const.tile([P, P], F16, tag="ones128")
        nc.vector.memset(ones128[:], 1.0)
        # multiplicative staircase masks (1 keep / 0 drop), applied post-exp
        stairs = []
        for j in range(4):
            st = const.tile([P, 512], F16, tag=f"stair{j}", name=f"st{j}")
            nc.vector.memset(st[:], 1.0)
            # keep 1 where f - p - 128j >= 0 else 0
            nc.gpsimd.affine_select(
                out=st[:], in_=st[:], compare_op=mybir.AluOpType.is_ge,
                fill=0.0, base=-128 * j, pattern=[[1, 512]],
                channel_multiplier=-1)
            stairs.append(st)
        bias_a_sb = const.tile([P, SC_A], F32, tag="bias_a")
        nc.sync.dma_start(bias_a_sb[:], bias_a[:])
        bias_b_sb = const.tile([P, SC_B], F32, tag="bias_b")
        nc.sync.dma_start(bias_b_sb[:], bias_b[:])
        eps_col = const.tile([P, 1], F32, tag="eps")
        nc.vector.memset(eps_col[:], EPS)

        # DRAM bounce buffers for the g-pair AllGathers
        dram = ctx.enter_context(tc.tile_pool(name="dram", bufs=1,
                                              space="DRAM"))
        cq_in = dram.tile([QL, 512], F16, tag="cq_in")
        cq_out = dram.tile([2 * QL, 512], F16, tag="cq_out")
        kv_in = dram.tile([KVL + ROPE, NQ], F16, tag="kv_in")
        kv_out = dram.tile([2 * (KVL + ROPE), NQ], F16, tag="kv_out")

        # persistent: kv latents + k_pe as two zero-padded K=128 variants
        latA = ctx.enter_context(tc.tile_pool(name="latA", bufs=1))
        kvT = [latA.tile([P, S], F16, tag=f"kvT{i}", name=f"kvT{i}")
               for i in range(NRKV)]
        kpe_e = latA.tile([P, S], F16, tag="kpe_e")
        kpe_o = latA.tile([P, S], F16, tag="kpe_o")
        nc.vector.memset(kpe_e[ROPE:P, :], 0.0)
        nc.vector.memset(kpe_o[0:ROPE, :], 0.0)
        kpez = (kpe_e, kpe_o)

        # packed cq latent: 24 [128,512] slices (rc, tbq) in 8 tiles;
        # reused as oTn after phase 3
        latQ = ctx.enter_context(tc.tile_pool(name="latQ", bufs=1))
        cqPk = [latQ.tile([P, 1536], F16, tag=f"cqPk{i}", name=f"cqPk{i}")
                for i in range(8)]

        def cq_slice(rc, tbq):
            idx = rc * 2 + tbq
            t, c = idx // 3, (idx % 3) * 512
            return cqPk[t][:, c:c + 512]

        oTn = [cqPk[h][:, 0:NQ] for h in range(HPC)]

        ps_main = ctx.enter_context(
            tc.tile_pool(name="ps_main", bufs=4, space="PSUM"))

        def mm_chain(ps_ap, pairs):
            n = len(pairs)
            for i, (lh, rh) in enumerate(pairs):
                nc.tensor.matmul(ps_ap, lh, rh,
                                 start=(i == 0), stop=(i == n - 1))

        ps_x_ctx = ExitStack()
        ps_x = ps_x_ctx.enter_context(
            tc.tile_pool(name="ps_x", bufs=2, space="PSUM"))

        # ---------- Phase 1: sharded A-projections + AllGathers ----------
        with ExitStack() as p1:
            tabk = p1.enter_context(tc.tile_pool(name="tabk", bufs=2))
            ropep = p1.enter_context(tc.tile_pool(name="ropep", bufs=2))
            wkva_p = p1.enter_context(tc.tile_pool(name="wkva", bufs=1))
            wqa_p = p1.enter_context(tc.tile_pool(name="wqa", bufs=1))
            wkva_sb = []
            for dc in range(ND):
                wt = wkva_p.tile([P, KVL + ROPE], F16, tag=f"wkva{dc}",
                                 name=f"wkva{dc}")
                nc.sync.dma_start(wt[:], wkva[dc * P:(dc + 1) * P, :])
                wkva_sb.append(wt)
            wqa_sb = []
            for dc in range(ND):
                wt = wqa_p.tile([P, QL], F16, tag=f"wqa{dc}",
                                name=f"wqa{dc}")
                nc.sync.dma_start(wt[:], wqa[dc * P:(dc + 1) * P, :])
                wqa_sb.append(wt)
            xt_p = p1.enter_context(tc.tile_pool(name="xt", bufs=2))
            sqp = p1.enter_context(tc.tile_pool(name="sq", bufs=3))

            def normalize(which, ntb):
                nrc, nfeat = ((NRKV, KVL) if which == 0 else (NRQ, QL))

                def sl_of(oc, tb):
                    if which == 0:
                        return kvT[oc][:, tb * 512:(tb + 1) * 512]
                    return cq_slice(oc, tb)
                for tb in range(ntb):
                    psb = ps_main.tile([P, 512], F32, tag="ps", name="psn")
                    for oc in range(nrc):
                        sq = sqp.tile([P, 512], F16, tag="sq", name="sq")
                        nc.scalar.activation(sq[:], sl_of(oc, tb), SQUARE)
                        nc.tensor.matmul(psb[:], ones128[:], sq[:],
                                         start=(oc == 0),
                                         stop=(oc == nrc - 1))
                    sd = sqp.tile([P, 512], F32, tag="sd", name="sd")
                    nc.scalar.activation(sd[:], psb[:], SQRT,
                                         bias=eps_col[:], scale=1.0 / nfeat)
                    rb = sqp.tile([P, 512], F32, tag="rb", name="rbn")
                    nc.vector.reciprocal_approx_fast(rb[:], sd[:])
                    for oc in range(nrc):
                        nc.vector.tensor_mul(sl_of(oc, tb), sl_of(oc, tb),
                                             rb[:])

            # KV A-proj first: this core's 1024 assigned tokens ->
            # kvT[:, 0:1024].  Its AllGather fires early so the hoisted
            # k/v up-projections can cover the cq AllGather afterwards.
            for hs in range(2):
                sl = slice(hs * 512, (hs + 1) * 512)
                xts = []
                for dc in range(ND):
                    xt = xt_p.tile([P, 512], F16, tag=f"xt{dc}",
                                   name=f"xt{dc}")
                    nc.sync.dma_start(xt[:], xkvT[dc * P:(dc + 1) * P, sl])
                    xts.append(xt)
                for oc in range(NRKV):
                    pool = ps_main if oc % 2 == 0 else ps_x
                    ps = pool.tile([P, 512], F32, tag="ps", name="ps1")
                    mm_chain(ps[:], [
                        (wkva_sb[dc][:, oc * P:(oc + 1) * P], xts[dc][:])
                        for dc in range(ND)])
                    if oc % 2 == 0:
                        nc.vector.tensor_copy(kvT[oc][:, sl], ps[:])
                    else:
                        nc.scalar.copy(kvT[oc][:, sl], ps[:])
                psp = ps_main.tile([ROPE, 512], F32, tag="ps", name="ps1p")
                mm_chain(psp[:], [
                    (wkva_sb[dc][:, KVL:KVL + ROPE], xts[dc][:])
                    for dc in range(ND)])
                ck = tabk.tile([HW, 512], F32, tag="cosk", name="ck")
                nc.sync.dma_start(ck[:], cosk[:, sl])
                sk = tabk.tile([HW, 512], F32, tag="sink", name="sk")
                nc.sync.dma_start(sk[:], sink[:, sl])
                _rope(nc, ropep, kpe_e[0:ROPE, sl], psp, ck[:], sk[:], 512)
            normalize(0, 2)
            # fire AG over the g-pair: kv latent + roped k_pe halves
            for oc in range(NRKV):
                nc.sync.dma_start(kv_in[oc * P:(oc + 1) * P, :],
                                  kvT[oc][:, 0:NQ])
            nc.sync.dma_start(kv_in[KVL:KVL + ROPE, :], kpe_e[0:ROPE, 0:NQ])
            nc.gpsimd.collective_compute(
                "AllGather", mybir.AluOpType.bypass, replica_groups=RG,
                ins=[kv_in[:].opt()], outs=[kv_out[:].opt()])

            # Q A-proj: this core's 512 assigned queries -> cq_slice(*, 0)
            xts = []
            for dc in range(ND):
                xt = xt_p.tile([P, 512], F16, tag=f"xt{dc}", name=f"xtq{dc}")
                nc.sync.dma_start(xt[:], xqT[dc * P:(dc + 1) * P, :])
                xts.append(xt)
            for oc in range(NRQ):
                pool = ps_main if oc % 2 == 0 else ps_x
                ps = pool.tile([P, 512], F32, tag="ps", name="ps1b")
                mm_chain(ps[:], [
                    (wqa_sb[dc][:, oc * P:(oc + 1) * P], xts[dc][:])
                    for dc in range(ND)])
                if oc % 2 == 0:
                    nc.vector.tensor_copy(cq_slice(oc, 0), ps[:])
                else:
                    nc.scalar.copy(cq_slice(oc, 0), ps[:])
            normalize(1, 1)
            # fire AG over the g-pair: cq halves (serializes after AG_kv on
            # the gpsimd queue; the hoisted k/v up-projections cover it)
            for rc in range(NRQ):
                nc.sync.dma_start(cq_in[rc * P:(rc + 1) * P, :],
                                  cq_slice(rc, 0))
            nc.gpsimd.collective_compute(
                "AllGather", mybir.AluOpType.bypass, replica_groups=RG,
                ins=[cq_in[:].opt()], outs=[cq_out[:].opt()])

            # read back the gathered kv latent + k_pe (both halves)
            for oc in range(NRKV):
                nc.sync.dma_start(kvT[oc][:, 0:NQ],
                                  kv_out[oc * P:(oc + 1) * P, :])
                nc.sync.dma_start(
                    kvT[oc][:, NQ:S],
                    kv_out[KVL + ROPE + oc * P:KVL + ROPE + (oc + 1) * P, :])
            nc.sync.dma_start(kpe_e[0:ROPE, 0:NQ], kv_out[KVL:KVL + ROPE, :])
            nc.sync.dma_start(kpe_e[0:ROPE, NQ:S],
                              kv_out[2 * KVL + ROPE:2 * (KVL + ROPE), :])
            nc.sync.dma_start(kpe_o[ROPE:P, :], kpe_e[0:ROPE, :])

        # ---- hoisted k/v up-projection (pairs 0,1): PE work that only
        # needs the kv AllGather, covering the cq AllGather's flight ----
        kt_p = ctx.enter_context(tc.tile_pool(name="kt", bufs=6))
        v_p = ctx.enter_context(tc.tile_pool(name="v", bufs=3))
        wk_p = ctx.enter_context(tc.tile_pool(name="wkvb", bufs=2))
        kTs = {}
        vts = {}

        def kv_up(hp2):
            heads2 = (2 * hp2, 2 * hp2 + 1)
            for h in heads2:
                wk_sb = []
                for rc in range(NRKV):
                    wt = wk_p.tile([P, NOPE], F16, tag=f"wkvbk{rc}",
                                   name=f"wkk{rc}")
                    nc.sync.dma_start(
                        wt[:], wkvbk[rc * P:(rc + 1) * P,
                                     h * NOPE:(h + 1) * NOPE])
                    wk_sb.append(wt)
                kt = kt_p.tile([P, S], F16, tag="kt", name=f"kt{h}")
                for tb in range(4):
                    sl = slice(tb * 512, (tb + 1) * 512)
                    ps = ps_main.tile([P, 512], F32, tag="ps", name="ps4k")
                    mm_chain(ps[:], [(wk_sb[rc][:], kvT[rc][:, sl])
                                     for rc in range(NRKV)])
                    if tb % 2 == 0:
                        nc.vector.tensor_copy(kt[:, sl], ps[:])
                    else:
                        nc.scalar.copy(kt[:, sl], ps[:])
                kTs[h] = kt
            wv_sb = []
            for rc in range(NRKV):
                wt = wk_p.tile([P, 2 * VD], F16, tag=f"wkvbv{rc}",
                               name=f"wkv{rc}")
                nc.sync.dma_start(
                    wt[:], wkvbv[rc * P:(rc + 1) * P,
                                 heads2[0] * VD:(heads2[0] + 2) * VD])
                wv_sb.append(wt)
            vt = v_p.tile([P, 16 * 2 * VD], F16, tag="vt", name=f"vt{hp2}")
            for tk in range(16):
                ps = ps_main.tile([P, 2 * VD], F32, tag="ps", name="ps4v")
                mm_chain(ps[:], [
                    (kvT[rc][:, tk * P:(tk + 1) * P], wv_sb[rc][:])
                    for rc in range(NRKV)])
                if tk % 2 == 0:
                    nc.vector.tensor_copy(
                        vt[:, tk * 2 * VD:(tk + 1) * 2 * VD], ps[:])
                else:
                    nc.scalar.copy(
                        vt[:, tk * 2 * VD:(tk + 1) * 2 * VD], ps[:])
            vts[hp2] = vt

        kv_up(0)
        kv_up(1)
        kv_up(2)

        # read back the gathered cq (issued after the k-up weight loads so
        # its cq-AllGather wait doesn't block them on the in-order queue)
        for rc in range(NRQ):
            nc.sync.dma_start(cq_slice(rc, 0),
                              cq_out[rc * P:(rc + 1) * P, :])
            nc.sync.dma_start(cq_slice(rc, 1),
                              cq_out[QL + rc * P:QL + (rc + 1) * P, :])

        # ---------- Phase 3: qT for all heads (head-paired rope) ----------
        latQT = ctx.enter_context(tc.tile_pool(name="latQT", bufs=1))
        qTn = [latQT.tile([P, NQ], F16, tag=f"qTn{h}", name=f"qTn{h}")
               for h in range(HPC)]
        qTpk = [latQT.tile([P, NQ], F16, tag=f"qTpk{i}", name=f"qTpk{i}")
                for i in range(HPC // 2)]
        with ExitStack() as p3:
            tabq = p3.enter_context(tc.tile_pool(name="tabq", bufs=1))
            cq_sb = tabq.tile([HW, NQ], F32, tag="cosq")
            nc.sync.dma_start(cq_sb[:], cosq[:])
            sq_sb = tabq.tile([HW, NQ], F32, tag="sinq")
            nc.sync.dma_start(sq_sb[:], sinq[:])
            ropep3 = p3.enter_context(tc.tile_pool(name="ropep3", bufs=2))
            wqb_p = p3.enter_context(tc.tile_pool(name="wqb", bufs=3))
            for hp in range(HPC // 2):
                wn, wr = [], []
                for rc in range(NRQ):
                    tn = wqb_p.tile([P, 2 * NOPE], F16, tag=f"wqbn{rc}",
                                    name=f"wqbn{rc}")
                    nc.sync.dma_start(
                        tn[:], wqbn[rc * P:(rc + 1) * P,
                                    hp * 2 * NOPE:(hp + 1) * 2 * NOPE])
                    wn.append(tn)
                    tr = wqb_p.tile([P, 2 * ROPE], F16, tag=f"wqbr{rc}",
                                    name=f"wqbr{rc}")
                    nc.sync.dma_start(
                        tr[:], wqbr[rc * P:(rc + 1) * P,
                                    hp * 2 * ROPE:(hp + 1) * 2 * ROPE])
                    wr.append(tr)
                for tbq in range(2):
                    sl = slice(tbq * 512, (tbq + 1) * 512)
                    for i in range(2):
                        h = 2 * hp + i
                        ps = ps_main.tile([P, 512], F32, tag="ps",
                                          name="ps3")
                        mm_chain(ps[:], [
                            (wn[rc][:, i * NOPE:(i + 1) * NOPE],
                             cq_slice(rc, tbq)) for rc in range(NRQ)])
                        if i == 0:
                            nc.vector.tensor_copy(qTn[h][:, sl], ps[:])
                        else:
                            nc.scalar.copy(qTn[h][:, sl], ps[:])
                    psp = ps_x.tile([P, 512], F32, tag="ps", name="ps3p")
                    mm_chain(psp[:], [(wr[rc][:], cq_slice(rc, tbq))
                                      for rc in range(NRQ)])
                    _rope(nc, ropep3, qTpk[hp][0:ROPE, sl], psp[0:ROPE, :],
                          cq_sb[:, sl], sq_sb[:, sl], 512)
                    _rope(nc, ropep3, qTpk[hp][ROPE:P, sl], psp[ROPE:P, :],
                          cq_sb[:, sl], sq_sb[:, sl], 512)

        ps_x_ctx.close()

        # ---------- Phase 4: attention per head-pair ----------
        wo_p = ctx.enter_context(tc.tile_pool(name="wo", bufs=1))
        wo_sb = []
        for h in range(HPC):
            wt = wo_p.tile([P, D], F16, tag=f"wo{h}", name=f"wo{h}")
            nc.sync.dma_start(wt[:], wo[h * P:(h + 1) * P, :])
            wo_sb.append(wt)
        with ExitStack() as p4:
            work = p4.enter_context(tc.tile_pool(name="work", bufs=4))
            accp = p4.enter_context(tc.tile_pool(name="accp", bufs=4))
            ptp = p4.enter_context(tc.tile_pool(name="ptp", bufs=28))
            ps_o = p4.enter_context(
                tc.tile_pool(name="ps_o", bufs=2, space="PSUM"))
            ps_d = p4.enter_context(
                tc.tile_pool(name="ps_d", bufs=2, space="PSUM"))
            for hp in range(HPC // 2):
                heads = (2 * hp, 2 * hp + 1)
                for h in heads:
                    hv = h % 2
                    pts = {0: [], 1: []}
                    # qb=0 denominator on DVE, qb=1 accumulated on the PE
                    acc0 = accp.tile([P, 512], F16, tag="acc0",
                                     name=f"acc{h}")
                    psb1 = ps_d.tile([P, 512], F32, tag="psd",
                                     name=f"psd{h}")
                    for sc in range(SC_B):
                        # both query blocks share each stationary load
                        sps = {}
                        for qb in ((0, 1) if sc < SC_A else (1,)):
                            sps[qb] = ps_main.tile([P, 512], F32, tag="ps",
                                                   name="ps4s")
                            nc.tensor.matmul(
                                sps[qb][:], kTs[h][:, sc * P:(sc + 1) * P],
                                qTn[h][:, qb * 512:qb * 512 + 512],
                                start=True, stop=False)
                        for qb in sps:
                            nc.tensor.matmul(
                                sps[qb][:],
                                kpez[hv][:, sc * P:(sc + 1) * P],
                                qTpk[h // 2][:, qb * 512:qb * 512 + 512],
                                start=False, stop=True)
                        for qb in sps:
                            nsc = SC_A if qb == 0 else SC_B
                            bias_sb = bias_a_sb if qb == 0 else bias_b_sb
                            pt = ptp.tile([P, 512], F16, tag="pt", name="pt")
                            jd = sc - (nsc - 4)
                            if jd >= 0:
                                nc.scalar.activation(pt[:], sps[qb][:], EXP)
                                nc.vector.tensor_mul(pt[:], pt[:],
                                                     stairs[jd][:])
                            else:
                                nc.scalar.activation(
                                    pt[:], sps[qb][:], EXP,
                                    bias=bias_sb[:, sc:sc + 1])
                            if qb == 0:
                                if sc == 0:
                                    nc.vector.tensor_copy(acc0[:], pt[:])
                                else:
                                    nc.vector.tensor_add(acc0[:], acc0[:],
                                                         pt[:])
                            else:
                                nc.tensor.matmul(psb1[:], ones128[:], pt[:],
                                                 start=(sc == 0),
                                                 stop=(sc == SC_B - 1))
                            pts[qb].append(pt)
                    oT = {qb: ps_o.tile([P, 512], F32, tag="oT",
                                        name=f"oT{qb}") for qb in (0, 1)}
                    for sc in range(SC_B):
                        for qb in ((0, 1) if sc < SC_A else (1,)):
                            nsc = SC_A if qb == 0 else SC_B
                            nc.tensor.matmul(
                                oT[qb][:],
                                vts[hp][:, sc * 2 * VD + hv * VD:
                                        sc * 2 * VD + (hv + 1) * VD],
                                pts[qb][sc][:], start=(sc == 0),
                                stop=(sc == nsc - 1))
                    psb0 = ps_main.tile([P, 512], F32, tag="ps", name="ps4d")
                    nc.tensor.matmul(psb0[:], ones128[:], acc0[:],
                                     start=True, stop=True)
                    for qb, psb in ((0, psb0), (1, psb1)):
                        rb = work.tile([P, 512], F32, tag="rb", name="rb")
                        nc.vector.reciprocal_approx_fast(rb[:], psb[:])
                        nc.vector.tensor_mul(
                            oTn[h][:, qb * 512:(qb + 1) * 512],
                            oT[qb][:], rb[:])
                # keep the in-order PE queue fed: the last pair's k/v
                # up-projection runs between the first attention pairs
                if hp == 0:
                    kv_up(3)

        # ---------- Phase 5: output projection (wo aliases qTn/kvT) -------
        with ExitStack() as p5:
            os_p = p5.enter_context(tc.tile_pool(name="os", bufs=4))
            for tk in range(NQ // P):
                for dcb in range(4):
                    ps = ps_main.tile([P, 512], F32, tag="ps", name="ps5")
                    for h in range(HPC):
                        rh = wo_sb[h][:, dcb * 512:(dcb + 1) * 512]
                        nc.tensor.matmul(
                            ps[:], oTn[h][:, tk * P:(tk + 1) * P], rh,
                            start=(h == 0), stop=(h == HPC - 1))
                    ot = os_p.tile([P, 512], F32, tag="ot", name="ot")
                    nc.scalar.copy(ot[:], ps[:])
                    nc.sync.dma_start(
                        out[tk * P:(tk + 1) * P,
                            dcb * 512:(dcb + 1) * 512], ot[:])

    nc.compile()
    return nc


def _prep_inputs(x, freqs_cis, wq_a, q_norm_w, wq_b, wkv_a, kv_norm_w,
                 wkv_b, wo):
    """Host-side shard prep. Returns (in_maps, meta) for 8 cores."""
    x = np.asarray(x, np.float32)
    freqs_cis = np.asarray(freqs_cis, np.float32)
    wq_a = np.asarray(wq_a, np.float32)
    q_norm_w = np.asarray(q_norm_w, np.float32)
    wq_b = np.asarray(wq_b, np.float32)
    wkv_a = np.asarray(wkv_a, np.float32)
    kv_norm_w = np.asarray(kv_norm_w, np.float32)
    wkv_b = np.asarray(wkv_b, np.float32)
    wo = np.asarray(wo, np.float32)

    f16 = np.float16
    # de-interleave perm for rope pairs: [e0..e31, o0..o31]
    perm = np.concatenate([np.arange(0, ROPE, 2), np.arange(1, ROPE, 2)])

    wqb = (wq_b * q_norm_w[:, None] * SCALE).reshape(QL, H, QKD)
    wqb_n = wqb[:, :, :NOPE].astype(f16)
    wqb_r = wqb[:, :, NOPE:][:, :, perm].astype(f16)

    wkva = np.ascontiguousarray(np.concatenate(
        [wkv_a[:, :KVL], wkv_a[:, KVL:][:, perm]], axis=1).astype(f16))

    wkvb = (wkv_b * kv_norm_w[:, None]).reshape(KVL, H, NOPE + VD).astype(f16)
    wkvb_k = wkvb[:, :, :NOPE]
    wkvb_v = wkvb[:, :, NOPE:]

    wqa16 = np.ascontiguousarray(wq_a.astype(f16))

    cos_t = np.ascontiguousarray(freqs_cis[:, :, 0].T)  # [32, S]
    sin_t = np.ascontiguousarray(freqs_cis[:, :, 1].T)

    sig0 = np.arange(S)
    sig1 = np.concatenate([sig0[512:1024], sig0[0:512],
                           sig0[1536:2048], sig0[1024:1536]])
    qpos = {0: np.concatenate([sig0[512:1024], sig0[1536:2048]]),
            1: np.concatenate([sig0[0:512], sig0[1024:1536]])}

    bias_a0 = np.zeros((P, SC_A), np.float32)
    bias_b0 = np.zeros((P, SC_B), np.float32)
    bias_a1 = np.zeros((P, SC_A), np.float32)
    bias_a1[:, 0:4] = NEG
    bias_b1 = np.zeros((P, SC_B), np.float32)
    bias_b1[:, 8:12] = NEG

    in_maps = []
    meta = []
    for c in range(N_CORES):
        b, g, t = c // 4, (c // 2) % 2, c % 2
        sig = sig0 if t == 0 else sig1
        qp = qpos[t]
        myq = qp[g * 512:(g + 1) * 512]      # assigned queries (global ids)
        mykv = sig[g * NQ:(g + 1) * NQ]      # assigned kv tokens
        hs = slice(g * HPC, (g + 1) * HPC)
        xb = x[b].T.astype(f16)
        m = {
            "xqT": np.ascontiguousarray(xb[:, myq]),
            "xkvT": np.ascontiguousarray(xb[:, mykv]),
            "wq_a": wqa16,
            "wq_b_n": np.ascontiguousarray(
                wqb_n[:, hs, :].reshape(QL, HPC * NOPE)),
            "wq_b_r": np.ascontiguousarray(
                wqb_r[:, hs, :].reshape(QL, HPC * ROPE)),
            "wkv_a": wkva,
            "wkv_b_k": np.ascontiguousarray(
                wkvb_k[:, hs, :].reshape(KVL, HPC * NOPE)),
            "wkv_b_v": np.ascontiguousarray(
                wkvb_v[:, hs, :].reshape(KVL, HPC * VD)),
            "wo": np.ascontiguousarray(
                wo[g * HPC * VD:(g + 1) * HPC * VD, :].astype(f16)),
            "cosq": np.ascontiguousarray(cos_t[:, qp]),
            "sinq": np.ascontiguousarray(sin_t[:, qp]),
            "cosk": np.ascontiguousarray(cos_t[:, mykv]),
            "sink": np.ascontiguousarray(sin_t[:, mykv]),
            "bias_a": bias_a0 if t == 0 else bias_a1,
            "bias_b": bias_b0 if t == 0 else bias_b1,
        }
        in_maps.append(m)
        meta.append((b, g, t))
    return in_maps, meta


def kernel(**inputs):
    in_maps, meta = _prep_inputs(**inputs)
    if "nc" not in _CACHE:
        _CACHE["nc"] = build_nc()
    nc = _CACHE["nc"]
    res = run_bass_kernel_spmd(nc, in_maps, core_ids=list(range(N_CORES)),
                               **_CACHE.get("run_kwargs", {}))
    _CACHE["last_result"] = res
    out = np.zeros((B, S, D), np.float32)
    for c in range(N_CORES):
        b, g, t = meta[c]
        part = res.results[c]["out"]  # [1024, 2048]
        if t == 0:
            out[b, 512:1024] += part[:512]
            out[b, 1536:2048] += part[512:]
        else:
            out[b, 0:512] += part[:512]
            out[b, 1024:1536] += part[512:]
    return out
